# revision 28
# baseline (speedup 1.0000x reference)
"""Trainium2 Bass kernel for nn_DistSelfAttention (Wasserstein self-attention).

Strategy: data-parallel over batch B=16 across 8 NeuronCores (2 batches/core,
no collectives). Per core, attention scores are computed TRANSPOSED
([Sk on partitions, Sq free]) so that:
  - the per-key -rk/8 Wasserstein term is a per-partition ACT bias (free),
  - the unnormalized prob matrix is directly the lhsT of the PV matmul,
  - the softmax denominator D rides along as a `ones` column in V (M=65).
Per-query-row Wasserstein terms (||mq_i||^2, sum cq_i) are dropped: softmax is
invariant to per-row constants.  exp() needs no max-subtraction: scores are
bounded above by ~(rq/8) <= O(30), safe in fp32.  The causal mask is applied
multiplicatively after exp (exp(mask) in {0,1} for 0/-1e4 masks; exact for any
mask as softmax(s+m) = (e^s * e^m)/sum).  zero_pad row 0 is folded into the
mask multiplier.  Matmuls run as float32r (full PE rate, ~fp32 precision).
"""

import os
import numpy as np

B, S, H, NH, DH = 16, 1024, 512, 8, 64
NCORES = 8
BL = B // NCORES          # batches per core
T = BL * S                # tokens per core
KC = H // 128             # 4 K-chunks of 128 features
NPAIR = NH // 2           # 4 head pairs

_CACHE = {}


def _build_program():
    import concourse.bass as bass
    import concourse.mybir as mybir
    import concourse.tile as tile
    from concourse import bacc
    from concourse.masks import make_identity

    FP = mybir.dt.float32
    FR = mybir.dt.float32r
    F8 = mybir.dt.float8e4
    BF = mybir.dt.bfloat16
    A = mybir.AluOpType
    AF = mybir.ActivationFunctionType

    nc = bacc.Bacc("TRN2", target_bir_lowering=False, debug=False,
                   num_devices=NCORES)

    def din(name, shape):
        return nc.dram_tensor(name, shape, FP, kind="ExternalInput").ap()

    def dout(name, shape):
        return nc.dram_tensor(name, shape, FP, kind="ExternalOutput").ap()

    xm = din("x_mean", [T, H])
    xrm = din("x_resp_mean", [T, H])
    xc = din("x_cov", [T, H])
    xrc = din("x_resp_cov", [T, H])
    mask = din("attn_mask", [S, S])
    wn = ["mq", "mk", "mv", "cq", "ck", "cv", "md", "cd"]
    Wd = {n: din(n + "_w", [H, H]).rearrange("(ko ki) n2 -> ki ko n2", ki=128)
          for n in wn}
    Bd = {n: din(n + "_b", [H]) for n in wn}
    lnw_d = din("ln_w", [H])
    lnb_d = din("ln_b", [H])
    out_mh = dout("mean_h", [T, H])
    out_ch = dout("cov_h", [T, H])
    out_p = dout("probs", [BL, NH, S, S])
    # internal DRAM scratch for per-head-normalized ctx^T (feature-major)
    ctx_dram = nc.dram_tensor("ctx_scratch", [BL, 2, H, S], FP).ap()

    def r(ap):
        return ap.bitcast(FR)

    def pbc_ap(dram_1d, parts):
        # partition-broadcast view of a 1-D DRAM tensor for DMA
        return bass.AP(tensor=dram_1d.tensor, offset=dram_1d.offset,
                       ap=[[0, parts]] + list(dram_1d.ap))

    with tile.TileContext(nc) as tc:
        import contextlib
        with contextlib.ExitStack() as ctx:
            singles = ctx.enter_context(tc.tile_pool(name="singles", bufs=1))
            bc = ctx.enter_context(tc.tile_pool(name="bc", bufs=5))
            big = ctx.enter_context(tc.tile_pool(name="big", bufs=4))
            combp = ctx.enter_context(tc.tile_pool(name="combp", bufs=3))
            vout = ctx.enter_context(tc.tile_pool(name="vout", bufs=1))
            wpool = ctx.enter_context(tc.tile_pool(name="wpool", bufs=4))
            wbigp = ctx.enter_context(tc.tile_pool(name="wbigp", bufs=2))
            sc = ctx.enter_context(tc.tile_pool(name="sc", bufs=10))
            ksqp = ctx.enter_context(tc.tile_pool(name="ksqp", bufs=1))
            dwp = ctx.enter_context(tc.tile_pool(name="dwp", bufs=2))
            tiny = ctx.enter_context(tc.tile_pool(name="tiny", bufs=4))
            pg = ctx.enter_context(
                tc.tile_pool(name="pg", bufs=2, space="PSUM"))
            pcm = ctx.enter_context(
                tc.tile_pool(name="pcm", bufs=2, space="PSUM"))
            pcc = ctx.enter_context(
                tc.tile_pool(name="pcc", bufs=2, space="PSUM"))
            ppt = ctx.enter_context(
                tc.tile_pool(name="ppt", bufs=2, space="PSUM"))

            # ---------------- setup ----------------
            ident = singles.tile([128, 128], FP, name="ident")
            make_identity(nc, ident)
            ident_fr = singles.tile([128, 128], FR, name="ident_fr")
            nc.vector.tensor_copy(ident_fr, ident)

            def tT(out_ps, in_sb):
                # fp32 transpose (inputs come straight from DMA, no rounding)
                nc.tensor.transpose(out_ps, in_sb, ident)

            def tTr(out_ps, in_sb):
                # f32r transpose (input already f32r-rounded)
                nc.tensor.transpose(r(out_ps), in_sb, ident_fr)

            ones_f = singles.tile([128, 1], FP, name="ones_f")
            nc.vector.memset(ones_f, 1.0)
            ones_col = singles.tile([128, 2], FR, name="ones_col")
            nc.vector.tensor_copy(ones_col,
                                  ones_f.to_broadcast((128, 2)))
            one_p0 = singles.tile([1, 2], FR, name="one_p0")
            nc.vector.tensor_copy(one_p0,
                                  ones_f[0:1, :].to_broadcast((1, 2)))
            ones_row = singles.tile([1, 128], FR, name="ones_row")
            nc.vector.tensor_copy(ones_row,
                                  ones_f[0:1, :].to_broadcast((1, 128)))
            # ones living at partition 64 (to pair with the D row there)
            ones65 = singles.tile([65, 128], FR, name="ones65")
            nc.vector.tensor_copy(ones65,
                                  ones_f[0:65, :].to_broadcast((65, 128)))
            eps_t = singles.tile([128, 1], FP, name="eps_t")
            nc.vector.memset(eps_t, 1e-12)

            # mT = 4*mask^T in bf16 (exact for 0/-1e4 masks); added into
            # the score PSUM before exp(0.25*psum - rk/8).  Column i=0 is
            # forced to -4e4 so exp zeroes probs row 0 (zero_pad).
            mT = singles.tile([128, 8, S], BF, name="mT")
            for q in range(2):          # i-half
                for jc in range(8):     # j-chunk
                    ps = pg.tile([128, 512], FP, tag="g", name="ps_mT")
                    for rr in range(4):  # i-chunk within half
                        mcol = sc.tile([128, 128], FP, tag="sc", name="mcol")
                        nc.sync.dma_start(
                            mcol,
                            mask[(4 * q + rr) * 128:(4 * q + rr + 1) * 128,
                                 jc * 128:(jc + 1) * 128])
                        tT(ps[:, rr * 128:(rr + 1) * 128], mcol)
                    nc.vector.tensor_scalar(mT[:, jc, q * 512:(q + 1) * 512],
                                            ps, 4.0, None, A.mult)
            nc.vector.memset(mT[:, :, 0:1], -4.0e4)

            # bias columns for q/k projections [128, NPAIR]
            bcol = {}
            for n in ("mq", "mk", "cq", "ck"):
                bt = singles.tile([128, NPAIR], FP, name=f"bcol_{n}")
                bcol[n] = bt
                for m in range(NPAIR):
                    if n in ("mq", "mk"):
                        nc.sync.dma_start(bt[:, m:m + 1],
                                          Bd[n][m * 128:(m + 1) * 128])
                    else:  # swapped head pairs (odd-head comb mirror)
                        nc.sync.dma_start(
                            bt[0:64, m:m + 1],
                            Bd[n][(2 * m + 1) * 64:(2 * m + 2) * 64])
                        nc.sync.dma_start(
                            bt[64:128, m:m + 1],
                            Bd[n][2 * m * 64:(2 * m + 1) * 64])

            def load_w_big(wdram, name):
                wv = wbigp.tile([128, KC, H], FR, tag="wbig", name=name)
                for k in range(KC):
                    wstg = sc.tile([128, 512], FP, tag="sc", name="wstg")
                    nc.sync.dma_start(wstg, wdram[:, k, :])
                    nc.gpsimd.tensor_copy(wv[:, k, :], wstg)
                return wv

            def bcast_row(dram_1d, name):
                t = bc.tile([128, H], FP, tag="bc", name=name)
                nc.sync.dma_start(t, pbc_ap(dram_1d, 128))
                return t

            def transpose_in(src, b, tag_name):
                """[1024, 512] slice of batch b -> feature-major [128,KC,S]."""
                xT = big.tile([128, KC, S], FR, tag="big", name=tag_name)
                for th in range(2):
                    xns = []
                    for tt_ in range(4):
                        t_ = 4 * th + tt_
                        xn = sc.tile([128, 512], FP, tag="sc", name="xn")
                        nc.sync.dma_start(
                            xn, src[b * S + t_ * 128: b * S + (t_ + 1) * 128, :])
                        xns.append(xn)
                    for c in range(KC):
                        ps = pg.tile([128, 512], FP, tag="g", name="ps_T")
                        for tt_ in range(4):
                            tT(ps[:, tt_ * 128:(tt_ + 1) * 128],
                               xns[tt_][:, c * 128:(c + 1) * 128])
                        nc.vector.tensor_copy(
                            xT[:, c, th * 512:(th + 1) * 512], ps)
                return xT

            def elu1(dst, src_ps, bias_col):
                """dst = elu(src+bias)+1 ;  dst, tmp in SBUF."""
                t0 = sc.tile([128, 512], FP, tag="sc", name="t0")
                t1 = sc.tile([128, 512], FP, tag="sc", name="t1")
                nc.vector.tensor_scalar(t0, src_ps, bias_col, None, A.add)
                nc.vector.tensor_scalar(t1, t0, 0.0, None, A.min)
                nc.scalar.activation(t1, t1, AF.Exp)
                nc.vector.tensor_scalar(t0, t0, 0.0, None, A.max)
                nc.vector.tensor_tensor(dst, t1, t0, A.add)

            for b in range(BL):
                # ---------------- V projections ----------------
                mvD = vout.tile([128, 8, NH, DH + 2], FR, tag="mvd",
                                name="mvD")
                nc.vector.tensor_copy(
                    mvD[:, :, :, DH:DH + 2],
                    ones_f.to_broadcast((128, 8, NH, 2)))
                cv_sb = vout.tile([128, 8, H], FR, tag="cvb", name="cv_sb")
                for src, n in ((xrm, "mv"), (xrc, "cv")):
                    xT = transpose_in(src, b, f"xT_{n}")
                    wv = load_w_big(Wd[n], f"w_{n}")
                    bb = bcast_row(Bd[n], f"bb_{n}")
                    for t_ in range(8):
                        ps = pg.tile([128, 512], FP, tag="g", name="ps_v")
                        for k in range(KC):
                            nc.tensor.matmul(
                                ps, xT[:, k, t_ * 128:(t_ + 1) * 128],
                                wv[:, k, :],
                                start=(k == 0), stop=(k == KC - 1))
                        if n == "mv":
                            nc.vector.tensor_tensor(
                                mvD[:, t_, :, 0:DH],
                                ps.rearrange("p (h d) -> p h d", d=DH),
                                bb.rearrange("p (h d) -> p h d", d=DH),
                                A.add)
                        else:
                            t0 = sc.tile([128, 512], FP, tag="sc", name="t0")
                            t1 = sc.tile([128, 512], FP, tag="sc", name="t1")
                            nc.vector.tensor_tensor(t0, ps, bb, A.add)
                            nc.vector.tensor_scalar(t1, t0, 0.0, None, A.min)
                            nc.scalar.activation(t1, t1, AF.Exp)
                            nc.vector.tensor_scalar(t0, t0, 0.0, None, A.max)
                            nc.vector.tensor_tensor(cv_sb[:, t_, :], t1, t0,
                                                    A.add)

                # ------------- Q/K projections + attention, per pair -------
                xTm = transpose_in(xm, b, "xTm")
                xTc = transpose_in(xc, b, "xTc")
                for m in range(NPAIR):
                    wsl = {}
                    for n in ("mq", "mk", "cq", "ck"):
                        wstg = sc.tile([128, KC, 128], FP, tag="sc",
                                       name="wslstg")
                        if n in ("mq", "mk"):
                            nc.sync.dma_start(wstg,
                                              Wd[n][:, :,
                                                    m * 128:(m + 1) * 128])
                        else:
                            nc.sync.dma_start(
                                wstg[:, :, 0:64],
                                Wd[n][:, :, (2 * m + 1) * 64:(2 * m + 2) * 64])
                            nc.sync.dma_start(
                                wstg[:, :, 64:128],
                                Wd[n][:, :, 2 * m * 64:(2 * m + 1) * 64])
                        w_ = wpool.tile([128, KC, 128], FR, tag="wsl",
                                        name=f"wsl_{n}")
                        wsl[n] = w_
                        nc.gpsimd.tensor_copy(w_, wstg)
                    combq = combp.tile([128, 2, S], FR, tag="comb",
                                       name="combq")
                    combk = combp.tile([128, 2, S], FR, tag="comb",
                                       name="combk")
                    cov_stage = []
                    for n2 in range(2):
                        sl = slice(n2 * 512, (n2 + 1) * 512)
                        # mean part
                        for nm, comb in (("mq", combq), ("mk", combk)):
                            ps = pg.tile([128, 512], FP, tag="g", name="ps_m")
                            for k in range(KC):
                                nc.tensor.matmul(
                                    ps, wsl[nm][:, k, :], xTm[:, k, sl],
                                    start=(k == 0), stop=(k == KC - 1))
                            nc.vector.tensor_scalar(
                                comb[0:64, 0, sl], ps[0:64],
                                bcol[nm][0:64, m:m + 1], None, A.add)
                            nc.vector.tensor_scalar(
                                comb[64:128, 1, sl], ps[64:128],
                                bcol[nm][64:128, m:m + 1], None, A.add)
                        # cov matmuls + bias add; elu/sqrt batched below so
                        # ACT runs its Exp (and later Sqrt) calls
                        # back-to-back without table reloads
                        for nq, comb in (("cq", combq), ("ck", combk)):
                            ps = pg.tile([128, 512], FP, tag="g", name="ps_c")
                            for k in range(KC):
                                nc.tensor.matmul(
                                    ps, wsl[nq][:, k, :], xTc[:, k, sl],
                                    start=(k == 0), stop=(k == KC - 1))
                            t0 = sc.tile([128, 512], FP, tag="sc", name="t0")
                            nc.scalar.add(t0, ps, bcol[nq][:, m:m + 1])
                            cov_stage.append((t0, comb, sl))
                    t1s = []
                    for t0, comb, sl in cov_stage:
                        t1 = sc.tile([128, 512], FP, tag="sc", name="t1")
                        nc.vector.tensor_scalar(t1, t0, 0.0, None, A.min)
                        t1s.append(t1)
                    for t1 in t1s:
                        nc.scalar.activation(t1, t1, AF.Exp)
                    for (t0, comb, sl), t1 in zip(cov_stage, t1s):
                        nc.vector.tensor_scalar(t0, t0, 0.0, None, A.max)
                        nc.vector.tensor_tensor(t1, t1, t0, A.add)
                    for (t0, comb, sl), t1 in zip(cov_stage, t1s):
                        nc.scalar.activation(t0, t1, AF.Sqrt)
                    for (t0, comb, sl), t1 in zip(cov_stage, t1s):
                        # centered cov features: d = sqrt(c) - 1
                        nc.vector.tensor_scalar(comb[64:128, 0, sl],
                                                t0[64:128], 1.0, None,
                                                A.subtract)
                        nc.vector.tensor_scalar(comb[0:64, 1, sl],
                                                t0[0:64], 1.0, None,
                                                A.subtract)

                    for p in range(2):
                        h = 2 * m + p
                        # rk columns: sum over 128 comb features of comb_k^2
                        # with centered cov features (d = sqrt(c)-1) the
                        # per-key bias is still just -(sum mk^2 + sum d^2)/8:
                        # the cross terms 2*sum(d_k) cancel exactly.
                        ksq = ksqp.tile([128, S], FR, tag="ksq", name="ksq")
                        nc.vector.tensor_tensor(ksq, combk[:, p, :],
                                                combk[:, p, :], A.mult)
                        rkps = ppt.tile([128, 16], FP, tag="pt", name="rkps")
                        for jc in range(8):
                            nc.tensor.matmul(
                                rkps[:, 2 * jc:2 * jc + 2],
                                ksq[:, jc * 128:(jc + 1) * 128],
                                ones_col, start=True, stop=True)
                        negrk = tiny.tile([128, 8], FP, tag="negrk",
                                          name="negrk")
                        nc.vector.tensor_scalar(
                            negrk,
                            rkps.rearrange("p (j t) -> p j t", t=2)[:, :, 0],
                            -0.125, None, A.mult)
                        negrk2 = tiny.tile([128, 8], FP, tag="negrk2",
                                           name="negrk2")
                        nc.vector.tensor_scalar(
                            negrk2,
                            rkps.rearrange("p (j t) -> p j t", t=2)[:, :, 0],
                            -0.25, None, A.mult)

                        for ih in range(2):          # Sq halves
                            isl = slice(ih * 512, (ih + 1) * 512)
                            pta = big.tile([128, 8, 512], FR, tag="big",
                                           name="pta")
                            cmps = pcm.tile([DH + 2, 512], FP, tag="cm",
                                            name="cmps")
                            ccps = pcc.tile([DH, 512], FP, tag="cc",
                                            name="ccps")
                            for jc in range(8):
                                g1 = pg.tile([128, 512], FP, tag="g",
                                             name="g1")
                                nc.tensor.matmul(
                                    g1,
                                    combk[:, p, jc * 128:(jc + 1) * 128],
                                    combq[:, p, isl],
                                    start=True, stop=True)
                                nc.vector.tensor_tensor(
                                    g1, g1, mT[:, jc, isl], A.add)
                                nc.scalar.activation(
                                    pta[:, jc, :], g1, AF.Exp,
                                    bias=negrk[:, jc:jc + 1], scale=0.25)
                                nc.tensor.matmul(
                                    cmps, mvD[:, jc, h, :], pta[:, jc, :],
                                    start=(jc == 0), stop=(jc == 7))
                                # P~^2 = exp(0.5*G - rk/4), same psum
                                sq = sc.tile([128, 512], FR, tag="sc",
                                             name="sq")
                                nc.scalar.activation(
                                    sq, g1, AF.Exp,
                                    bias=negrk2[:, jc:jc + 1], scale=0.5)
                                nc.tensor.matmul(
                                    ccps,
                                    cv_sb[:, jc, h * DH:(h + 1) * DH],
                                    sq, start=(jc == 0), stop=(jc == 7))
                            # D row -> clamp -> 1/D (row form, cheap) ->
                            # PE-broadcast invD and invD^2 to 64 partitions
                            dxt = sc.tile([DH + 1, 512], FP, tag="sc",
                                          name="dxt")
                            nc.vector.tensor_copy(dxt[DH:DH + 1, :],
                                                  cmps[DH:DH + 1, :])
                            nc.vector.tensor_scalar(dxt[DH:DH + 1, :],
                                                    dxt[DH:DH + 1, :],
                                                    1e-15, None, A.max)
                            invb = dwp.tile([DH + 1, 1024], FP, tag="dw",
                                            name="invb")
                            nc.vector.reciprocal(invb[DH:DH + 1, 0:512],
                                                 dxt[DH:DH + 1, :])
                            nc.vector.tensor_tensor(invb[DH:DH + 1, 512:],
                                                    invb[DH:DH + 1, 0:512],
                                                    invb[DH:DH + 1, 0:512],
                                                    A.mult)
                            invf = dwp.tile([DH + 1, 1024], FR, tag="dw",
                                            name="invf")
                            nc.vector.tensor_copy(invf[DH:DH + 1, :],
                                                  invb[DH:DH + 1, :])
                            ibp = ppt.tile([64, 512], FP, tag="pt",
                                           name="ibp")
                            nc.tensor.matmul(ibp, ones65[64:65, 0:64],
                                             invf[DH:DH + 1, 0:512],
                                             start=True, stop=True)
                            ibc = sc.tile([64, 512], FP, tag="sc", name="ibc")
                            nc.scalar.copy(ibc, ibp)
                            ibp2 = ppt.tile([64, 512], FP, tag="pt",
                                            name="ibp2")
                            nc.tensor.matmul(ibp2, ones65[64:65, 0:64],
                                             invf[DH:DH + 1, 512:],
                                             start=True, stop=True)
                            ibc2 = sc.tile([64, 512], FP, tag="sc",
                                           name="ibc2")
                            nc.scalar.copy(ibc2, ibp2)
                            # ctx eviction + normalization -> DRAM scratch
                            fsl = slice(m * 128 + p * 64, m * 128 + p * 64 + 64)
                            tsl = slice(ih * 512, ih * 512 + 512)
                            cm_sb = sc.tile([64, 512], FP, tag="sc",
                                            name="cm_sb")
                            nc.vector.tensor_tensor(cm_sb, cmps[0:DH],
                                                    ibc[0:64], A.mult)
                            nc.sync.dma_start(ctx_dram[b, 0, fsl, tsl], cm_sb)
                            cc_sb = sc.tile([64, 512], FP, tag="sc",
                                            name="cc_sb")
                            nc.vector.tensor_tensor(cc_sb, ccps[0:DH],
                                                    ibc2[0:64], A.mult)
                            nc.sync.dma_start(ctx_dram[b, 1, fsl, tsl], cc_sb)
                            # invD columns for probs normalization
                            icols = tiny.tile([128, 4], FP, tag="ic",
                                              name="icols")
                            for c4 in range(4):
                                nc.sync.dma_start(
                                    icols[:, c4:c4 + 1],
                                    invb[DH:DH + 1,
                                         c4 * 128:(c4 + 1) * 128])
                            # transpose P~t -> natural probs, normalize, store
                            for jq in range(2):
                                for c4 in range(4):
                                    ptp = ppt.tile([128, 512], FP, tag="pt",
                                                   name="ptp")
                                    for jj in range(4):
                                        tTr(ptp[:, jj * 128:(jj + 1) * 128],
                                            pta[:, 4 * jq + jj,
                                                c4 * 128:(c4 + 1) * 128])
                                    pn = sc.tile([128, 512], FP, tag="sc",
                                                 name="pn")
                                    if c4 % 2 == 0:
                                        nc.vector.tensor_scalar(
                                            pn, ptp, icols[:, c4:c4 + 1],
                                            None, A.mult)
                                    else:
                                        nc.scalar.mul(pn, ptp,
                                                      icols[:, c4:c4 + 1])
                                    nc.sync.dma_start(
                                        out_p[b, h,
                                              ih * 512 + c4 * 128:
                                              ih * 512 + (c4 + 1) * 128,
                                              jq * 512:(jq + 1) * 512],
                                        pn)

                # ---------------- dense + layernorm ----------------
                lnw_bc = bcast_row(lnw_d, "lnw_bc")
                lnb_bc = bcast_row(lnb_d, "lnb_bc")
                for path, (wname, xres, outd) in enumerate(
                        (("md", xm, out_mh), ("cd", xc, out_ch))):
                    wd_ = load_w_big(Wd[wname], f"w_{wname}")
                    bb = bcast_row(Bd[wname], f"bb_{wname}")
                    ctxT = big.tile([128, KC, S], FR, tag="big", name="ctxT")
                    ctx_v = ctx_dram[b, path].rearrange(
                        "(ko ki) s -> ki ko s", ki=128)
                    for k in range(KC):
                        cstg = ksqp.tile([128, S], FP, tag="ksq",
                                         name="cstg")
                        nc.sync.dma_start(cstg, ctx_v[:, k, :])
                        nc.gpsimd.tensor_copy(ctxT[:, k, :], cstg)
                    for t_ in range(8):
                        ps = pg.tile([128, 512], FP, tag="g", name="ps_d")
                        for k in range(KC):
                            nc.tensor.matmul(
                                ps, ctxT[:, k, t_ * 128:(t_ + 1) * 128],
                                wd_[:, k, :],
                                start=(k == 0), stop=(k == KC - 1))
                        res = sc.tile([128, 512], FP, tag="sc", name="res")
                        nc.sync.dma_start(
                            res,
                            xres[b * S + t_ * 128: b * S + (t_ + 1) * 128, :])
                        s1 = sc.tile([128, 512], FP, tag="sc", name="s1")
                        nc.vector.tensor_tensor(s1, ps, bb, A.add)
                        nc.vector.tensor_tensor(s1, s1, res, A.add)
                        stats = tiny.tile([128, 6], FP, tag="st",
                                          name="stats")
                        nc.vector.bn_stats(stats, s1)
                        mv2 = tiny.tile([128, 2], FP, tag="mv2", name="mv2")
                        nc.vector.bn_aggr(mv2, stats)
                        std = tiny.tile([128, 1], FP, tag="std", name="std")
                        nc.scalar.activation(std, mv2[:, 1:2], AF.Sqrt,
                                             bias=eps_t)
                        rstd = tiny.tile([128, 1], FP, tag="rstd",
                                         name="rstd")
                        nc.vector.reciprocal(rstd, std)
                        s2 = sc.tile([128, 512], FP, tag="sc", name="s2")
                        nc.vector.tensor_scalar(s2, s1, mv2[:, 0:1], rstd,
                                                A.subtract, A.mult)
                        nc.vector.tensor_tensor(s2, s2, lnw_bc, A.mult)
                        nc.vector.tensor_tensor(s2, s2, lnb_bc, A.add)
                        nc.sync.dma_start(
                            outd[b * S + t_ * 128: b * S + (t_ + 1) * 128, :],
                            s2)
    nc.compile()
    return nc


def _get_program():
    if "nc" not in _CACHE:
        _CACHE["nc"] = _build_program()
    return _CACHE["nc"]


def kernel(**inputs):
    from concourse.bass_utils import run_bass_kernel_spmd

    nc = _get_program()
    in_maps = []
    for c in range(NCORES):
        bsl = slice(c * BL, (c + 1) * BL)
        m = {
            "x_mean": np.ascontiguousarray(
                inputs["x_mean"][bsl].reshape(T, H)),
            "x_resp_mean": np.ascontiguousarray(
                inputs["x_resp_mean"][bsl].reshape(T, H)),
            "x_cov": np.ascontiguousarray(
                inputs["x_cov"][bsl].reshape(T, H)),
            "x_resp_cov": np.ascontiguousarray(
                inputs["x_resp_cov"][bsl].reshape(T, H)),
            "attn_mask": np.ascontiguousarray(
                inputs["attn_mask"].reshape(S, S)),
            "ln_w": inputs["ln_w"], "ln_b": inputs["ln_b"],
        }
        for n in ["mq", "mk", "mv", "cq", "ck", "cv", "md", "cd"]:
            m[n + "_w"] = inputs[n + "_w"]
            m[n + "_b"] = inputs[n + "_b"]
        in_maps.append(m)

    res = run_bass_kernel_spmd(nc, in_maps, list(range(NCORES))).results

    mean_h = np.concatenate(
        [res[c]["mean_h"].reshape(BL, S, H) for c in range(NCORES)], axis=0)
    cov_h = np.concatenate(
        [res[c]["cov_h"].reshape(BL, S, H) for c in range(NCORES)], axis=0)
    probs = np.concatenate(
        [res[c]["probs"] for c in range(NCORES)], axis=0)
    return mean_h, cov_h, probs


# revision 29
# speedup vs baseline: 1.1477x; 1.1477x over previous
"""Trainium2 Bass kernel for nn_DistSelfAttention (Wasserstein self-attention).

Strategy: data-parallel over batch B=16 across 8 NeuronCores (2 batches/core,
no collectives). Per core, attention scores are computed TRANSPOSED
([Sk on partitions, Sq free]) so that:
  - the per-key -rk/8 Wasserstein term is a per-partition ACT bias (free),
  - the unnormalized prob matrix is directly the lhsT of the PV matmul,
  - the softmax denominator D rides along as a `ones` column in V (M=65).
Per-query-row Wasserstein terms (||mq_i||^2, sum cq_i) are dropped: softmax is
invariant to per-row constants.  exp() needs no max-subtraction: scores are
bounded above by ~(rq/8) <= O(30), safe in fp32.  The causal mask is applied
multiplicatively after exp (exp(mask) in {0,1} for 0/-1e4 masks; exact for any
mask as softmax(s+m) = (e^s * e^m)/sum).  zero_pad row 0 is folded into the
mask multiplier.  Matmuls run as float32r (full PE rate, ~fp32 precision).
"""

import os
import numpy as np

B, S, H, NH, DH = 16, 1024, 512, 8, 64
NCORES = 8
BL = B // NCORES          # batches per core
T = BL * S                # tokens per core
KC = H // 128             # 4 K-chunks of 128 features
NPAIR = NH // 2           # 4 head pairs

_CACHE = {}


def _build_program():
    import concourse.bass as bass
    import concourse.mybir as mybir
    import concourse.tile as tile
    from concourse import bacc
    from concourse.masks import make_identity

    FP = mybir.dt.float32
    FR = mybir.dt.float32r
    F8 = mybir.dt.float8e4
    BF = mybir.dt.bfloat16
    A = mybir.AluOpType
    AF = mybir.ActivationFunctionType

    nc = bacc.Bacc("TRN2", target_bir_lowering=False, debug=False,
                   num_devices=NCORES)

    def din(name, shape):
        return nc.dram_tensor(name, shape, FP, kind="ExternalInput").ap()

    def dout(name, shape):
        return nc.dram_tensor(name, shape, FP, kind="ExternalOutput").ap()

    xm = din("x_mean", [T, H])
    xrm = din("x_resp_mean", [T, H])
    xc = din("x_cov", [T, H])
    xrc = din("x_resp_cov", [T, H])
    mask = din("attn_mask", [S, S])
    wn = ["mq", "mk", "mv", "cq", "ck", "cv", "md", "cd"]
    Wd = {n: din(n + "_w", [H, H]).rearrange("(ko ki) n2 -> ki ko n2", ki=128)
          for n in wn}
    Bd = {n: din(n + "_b", [H]) for n in wn}
    lnw_d = din("ln_w", [H])
    lnb_d = din("ln_b", [H])
    out_mh = dout("mean_h", [T, H])
    out_ch = dout("cov_h", [T, H])
    out_p = dout("probs", [BL, NH, S, S])
    # internal DRAM scratch for per-head-normalized ctx^T (feature-major)
    ctx_dram = nc.dram_tensor("ctx_scratch", [BL, 2, H, S], FP).ap()

    def r(ap):
        return ap.bitcast(FR)

    def pbc_ap(dram_1d, parts):
        # partition-broadcast view of a 1-D DRAM tensor for DMA
        return bass.AP(tensor=dram_1d.tensor, offset=dram_1d.offset,
                       ap=[[0, parts]] + list(dram_1d.ap))

    with tile.TileContext(nc) as tc:
        import contextlib
        with contextlib.ExitStack() as ctx:
            singles = ctx.enter_context(tc.tile_pool(name="singles", bufs=1))
            bc = ctx.enter_context(tc.tile_pool(name="bc", bufs=5))
            big = ctx.enter_context(tc.tile_pool(name="big", bufs=4))
            combp = ctx.enter_context(tc.tile_pool(name="combp", bufs=3))
            vout = ctx.enter_context(tc.tile_pool(name="vout", bufs=1))
            wpool = ctx.enter_context(tc.tile_pool(name="wpool", bufs=4))
            wbigp = ctx.enter_context(tc.tile_pool(name="wbigp", bufs=2))
            sc = ctx.enter_context(tc.tile_pool(name="sc", bufs=10))
            ksqp = ctx.enter_context(tc.tile_pool(name="ksqp", bufs=1))
            dwp = ctx.enter_context(tc.tile_pool(name="dwp", bufs=2))
            tiny = ctx.enter_context(tc.tile_pool(name="tiny", bufs=4))
            pg = ctx.enter_context(
                tc.tile_pool(name="pg", bufs=2, space="PSUM"))
            pcm = ctx.enter_context(
                tc.tile_pool(name="pcm", bufs=2, space="PSUM"))
            pcc = ctx.enter_context(
                tc.tile_pool(name="pcc", bufs=2, space="PSUM"))
            ppt = ctx.enter_context(
                tc.tile_pool(name="ppt", bufs=2, space="PSUM"))

            # ---------------- setup ----------------
            ident = singles.tile([128, 128], FP, name="ident")
            make_identity(nc, ident)
            ident_fr = singles.tile([128, 128], FR, name="ident_fr")
            nc.vector.tensor_copy(ident_fr, ident)

            def tT(out_ps, in_sb):
                # fp32 transpose (inputs come straight from DMA, no rounding)
                nc.tensor.transpose(out_ps, in_sb, ident)

            def tTr(out_ps, in_sb):
                # f32r transpose (input already f32r-rounded)
                nc.tensor.transpose(r(out_ps), in_sb, ident_fr)

            ones_f = singles.tile([128, 1], FP, name="ones_f")
            nc.vector.memset(ones_f, 1.0)
            ones_col = singles.tile([128, 2], FR, name="ones_col")
            nc.vector.tensor_copy(ones_col,
                                  ones_f.to_broadcast((128, 2)))
            one_p0 = singles.tile([1, 2], FR, name="one_p0")
            nc.vector.tensor_copy(one_p0,
                                  ones_f[0:1, :].to_broadcast((1, 2)))
            ones_row = singles.tile([1, 128], FR, name="ones_row")
            nc.vector.tensor_copy(ones_row,
                                  ones_f[0:1, :].to_broadcast((1, 128)))
            # ones living at partition 64 (to pair with the D row there)
            ones65 = singles.tile([65, 128], FR, name="ones65")
            nc.vector.tensor_copy(ones65,
                                  ones_f[0:65, :].to_broadcast((65, 128)))
            eps_t = singles.tile([128, 1], FP, name="eps_t")
            nc.vector.memset(eps_t, 1e-12)

            # mT = 4*mask^T in bf16 (exact for 0/-1e4 masks); added into
            # the score PSUM before exp(0.25*psum - rk/8).  Column i=0 is
            # forced to -4e4 so exp zeroes probs row 0 (zero_pad).
            mT = singles.tile([128, 8, S], BF, name="mT")
            for q in range(2):          # i-half
                for jc in range(8):     # j-chunk
                    ps = pg.tile([128, 512], FP, tag="g", name="ps_mT")
                    for rr in range(4):  # i-chunk within half
                        mcol = sc.tile([128, 128], FP, tag="sc", name="mcol")
                        nc.sync.dma_start(
                            mcol,
                            mask[(4 * q + rr) * 128:(4 * q + rr + 1) * 128,
                                 jc * 128:(jc + 1) * 128])
                        tT(ps[:, rr * 128:(rr + 1) * 128], mcol)
                    nc.vector.tensor_scalar(mT[:, jc, q * 512:(q + 1) * 512],
                                            ps, 4.0, None, A.mult)
            nc.vector.memset(mT[:, :, 0:1], -4.0e4)

            # bias columns for q/k projections [128, NPAIR]
            bcol = {}
            for n in ("mq", "mk", "cq", "ck"):
                bt = singles.tile([128, NPAIR], FP, name=f"bcol_{n}")
                bcol[n] = bt
                for m in range(NPAIR):
                    if n in ("mq", "mk"):
                        nc.sync.dma_start(bt[:, m:m + 1],
                                          Bd[n][m * 128:(m + 1) * 128])
                    else:  # swapped head pairs (odd-head comb mirror)
                        nc.sync.dma_start(
                            bt[0:64, m:m + 1],
                            Bd[n][(2 * m + 1) * 64:(2 * m + 2) * 64])
                        nc.sync.dma_start(
                            bt[64:128, m:m + 1],
                            Bd[n][2 * m * 64:(2 * m + 1) * 64])

            def load_w_big(wdram, name):
                wv = wbigp.tile([128, KC, H], FR, tag="wbig", name=name)
                for k in range(KC):
                    wstg = sc.tile([128, 512], FP, tag="sc", name="wstg")
                    nc.sync.dma_start(wstg, wdram[:, k, :])
                    nc.gpsimd.tensor_copy(wv[:, k, :], wstg)
                return wv

            def bcast_row(dram_1d, name):
                t = bc.tile([128, H], FP, tag="bc", name=name)
                nc.sync.dma_start(t, pbc_ap(dram_1d, 128))
                return t

            def transpose_in(src, b, tag_name):
                """[1024, 512] slice of batch b -> feature-major [128,KC,S]."""
                xT = big.tile([128, KC, S], FR, tag="big", name=tag_name)
                for th in range(2):
                    xns = []
                    for tt_ in range(4):
                        t_ = 4 * th + tt_
                        xn = sc.tile([128, 512], FP, tag="sc", name="xn")
                        nc.sync.dma_start(
                            xn, src[b * S + t_ * 128: b * S + (t_ + 1) * 128, :])
                        xns.append(xn)
                    for c in range(KC):
                        ps = pg.tile([128, 512], FP, tag="g", name="ps_T")
                        for tt_ in range(4):
                            tT(ps[:, tt_ * 128:(tt_ + 1) * 128],
                               xns[tt_][:, c * 128:(c + 1) * 128])
                        nc.vector.tensor_copy(
                            xT[:, c, th * 512:(th + 1) * 512], ps)
                return xT

            def elu1(dst, src_ps, bias_col):
                """dst = elu(src+bias)+1 ;  dst, tmp in SBUF."""
                t0 = sc.tile([128, 512], FP, tag="sc", name="t0")
                t1 = sc.tile([128, 512], FP, tag="sc", name="t1")
                nc.vector.tensor_scalar(t0, src_ps, bias_col, None, A.add)
                nc.vector.tensor_scalar(t1, t0, 0.0, None, A.min)
                nc.scalar.activation(t1, t1, AF.Exp)
                nc.vector.tensor_scalar(t0, t0, 0.0, None, A.max)
                nc.vector.tensor_tensor(dst, t1, t0, A.add)

            for b in range(BL):
                # ---------------- V projections ----------------
                mvD = vout.tile([128, 8, NH, DH + 2], FR, tag="mvd",
                                name="mvD")
                nc.vector.tensor_copy(
                    mvD[:, :, :, DH:DH + 2],
                    ones_f.to_broadcast((128, 8, NH, 2)))
                cv_sb = vout.tile([128, 8, H], FR, tag="cvb", name="cv_sb")
                for src, n in ((xrm, "mv"), (xrc, "cv")):
                    xT = transpose_in(src, b, f"xT_{n}")
                    wv = load_w_big(Wd[n], f"w_{n}")
                    bb = bcast_row(Bd[n], f"bb_{n}")
                    for t_ in range(8):
                        ps = pg.tile([128, 512], FP, tag="g", name="ps_v")
                        for k in range(KC):
                            nc.tensor.matmul(
                                ps, xT[:, k, t_ * 128:(t_ + 1) * 128],
                                wv[:, k, :],
                                start=(k == 0), stop=(k == KC - 1))
                        if n == "mv":
                            nc.vector.tensor_tensor(
                                mvD[:, t_, :, 0:DH],
                                ps.rearrange("p (h d) -> p h d", d=DH),
                                bb.rearrange("p (h d) -> p h d", d=DH),
                                A.add)
                        else:
                            t0 = sc.tile([128, 512], FP, tag="sc", name="t0")
                            t1 = sc.tile([128, 512], FP, tag="sc", name="t1")
                            nc.vector.tensor_tensor(t0, ps, bb, A.add)
                            nc.vector.tensor_scalar(t1, t0, 0.0, None, A.min)
                            nc.scalar.activation(t1, t1, AF.Exp)
                            nc.vector.tensor_scalar(t0, t0, 0.0, None, A.max)
                            nc.vector.tensor_tensor(cv_sb[:, t_, :], t1, t0,
                                                    A.add)

                # ------------- Q/K projections + attention, per pair -------
                xTm = transpose_in(xm, b, "xTm")
                xTc = transpose_in(xc, b, "xTc")
                for m in range(NPAIR):
                    wsl = {}
                    for n in ("mq", "mk", "cq", "ck"):
                        wstg = sc.tile([128, KC, 128], FP, tag="sc",
                                       name="wslstg")
                        if n in ("mq", "mk"):
                            nc.sync.dma_start(wstg,
                                              Wd[n][:, :,
                                                    m * 128:(m + 1) * 128])
                        else:
                            nc.sync.dma_start(
                                wstg[:, :, 0:64],
                                Wd[n][:, :, (2 * m + 1) * 64:(2 * m + 2) * 64])
                            nc.sync.dma_start(
                                wstg[:, :, 64:128],
                                Wd[n][:, :, 2 * m * 64:(2 * m + 1) * 64])
                        w_ = wpool.tile([128, KC, 128], FR, tag="wsl",
                                        name=f"wsl_{n}")
                        wsl[n] = w_
                        nc.gpsimd.tensor_copy(w_, wstg)
                    combq = combp.tile([128, 2, S], FR, tag="comb",
                                       name="combq")
                    combk = combp.tile([128, 2, S], FR, tag="comb",
                                       name="combk")
                    cov_stage = []
                    for n2 in range(2):
                        sl = slice(n2 * 512, (n2 + 1) * 512)
                        # mean part
                        for nm, comb in (("mq", combq), ("mk", combk)):
                            ps = pg.tile([128, 512], FP, tag="g", name="ps_m")
                            for k in range(KC):
                                nc.tensor.matmul(
                                    ps, wsl[nm][:, k, :], xTm[:, k, sl],
                                    start=(k == 0), stop=(k == KC - 1))
                            nc.vector.tensor_scalar(
                                comb[0:64, 0, sl], ps[0:64],
                                bcol[nm][0:64, m:m + 1], None, A.add)
                            nc.vector.tensor_scalar(
                                comb[64:128, 1, sl], ps[64:128],
                                bcol[nm][64:128, m:m + 1], None, A.add)
                        # cov matmuls + bias add; elu/sqrt batched below so
                        # ACT runs its Exp (and later Sqrt) calls
                        # back-to-back without table reloads
                        for nq, comb in (("cq", combq), ("ck", combk)):
                            ps = pg.tile([128, 512], FP, tag="g", name="ps_c")
                            for k in range(KC):
                                nc.tensor.matmul(
                                    ps, wsl[nq][:, k, :], xTc[:, k, sl],
                                    start=(k == 0), stop=(k == KC - 1))
                            t0 = sc.tile([128, 512], FP, tag="sc", name="t0")
                            nc.scalar.add(t0, ps, bcol[nq][:, m:m + 1])
                            cov_stage.append((t0, comb, sl))
                    t1s = []
                    for t0, comb, sl in cov_stage:
                        t1 = sc.tile([128, 512], FP, tag="sc", name="t1")
                        nc.vector.tensor_scalar(t1, t0, 0.0, None, A.min)
                        t1s.append(t1)
                    for t1 in t1s:
                        nc.scalar.activation(t1, t1, AF.Exp)
                    for (t0, comb, sl), t1 in zip(cov_stage, t1s):
                        nc.vector.tensor_scalar(t0, t0, 0.0, None, A.max)
                        nc.vector.tensor_tensor(t1, t1, t0, A.add)
                    for (t0, comb, sl), t1 in zip(cov_stage, t1s):
                        nc.scalar.activation(t0, t1, AF.Sqrt)
                    for (t0, comb, sl), t1 in zip(cov_stage, t1s):
                        # centered cov features: d = sqrt(c) - 1
                        nc.vector.tensor_scalar(comb[64:128, 0, sl],
                                                t0[64:128], 1.0, None,
                                                A.subtract)
                        nc.vector.tensor_scalar(comb[0:64, 1, sl],
                                                t0[0:64], 1.0, None,
                                                A.subtract)

                    for p in range(2):
                        h = 2 * m + p
                        # rk columns: sum over 128 comb features of comb_k^2
                        # with centered cov features (d = sqrt(c)-1) the
                        # per-key bias is still just -(sum mk^2 + sum d^2)/8:
                        # the cross terms 2*sum(d_k) cancel exactly.
                        ksq = ksqp.tile([128, S], FR, tag="ksq", name="ksq")
                        nc.vector.tensor_tensor(ksq, combk[:, p, :],
                                                combk[:, p, :], A.mult)
                        rkps = ppt.tile([128, 16], FP, tag="pt", name="rkps")
                        for jc in range(8):
                            nc.tensor.matmul(
                                rkps[:, 2 * jc:2 * jc + 2],
                                ksq[:, jc * 128:(jc + 1) * 128],
                                ones_col, start=True, stop=True)
                        negrk = tiny.tile([128, 8], FP, tag="negrk",
                                          name="negrk")
                        nc.vector.tensor_scalar(
                            negrk,
                            rkps.rearrange("p (j t) -> p j t", t=2)[:, :, 0],
                            -0.125, None, A.mult)
                        negrk2 = tiny.tile([128, 8], FP, tag="negrk2",
                                           name="negrk2")
                        nc.vector.tensor_scalar(
                            negrk2,
                            rkps.rearrange("p (j t) -> p j t", t=2)[:, :, 0],
                            -0.25, None, A.mult)

                        for ih in range(2):          # Sq halves
                            isl = slice(ih * 512, (ih + 1) * 512)
                            pta = big.tile([128, 8, 512], FR, tag="big",
                                           name="pta")
                            cmps = pcm.tile([DH + 2, 512], FP, tag="cm",
                                            name="cmps")
                            ccps = pcc.tile([DH, 512], FP, tag="cc",
                                            name="ccps")
                            for jc in range(8):
                                g1 = pg.tile([128, 512], FP, tag="g",
                                             name="g1")
                                nc.tensor.matmul(
                                    g1,
                                    combk[:, p, jc * 128:(jc + 1) * 128],
                                    combq[:, p, isl],
                                    start=True, stop=True)
                                nc.vector.tensor_tensor(
                                    g1, g1, mT[:, jc, isl], A.add)
                                nc.scalar.activation(
                                    pta[:, jc, :], g1, AF.Exp,
                                    bias=negrk[:, jc:jc + 1], scale=0.25)
                                nc.tensor.matmul(
                                    cmps, mvD[:, jc, h, :], pta[:, jc, :],
                                    start=(jc == 0), stop=(jc == 7))
                            for jc in range(8):
                                sq = sc.tile([128, 512], FR, tag="sc",
                                             name="sq")
                                nc.gpsimd.tensor_tensor(sq, pta[:, jc, :],
                                                        pta[:, jc, :],
                                                        A.mult)
                                nc.tensor.matmul(
                                    ccps,
                                    cv_sb[:, jc, h * DH:(h + 1) * DH],
                                    sq, start=(jc == 0), stop=(jc == 7))
                            # D row -> clamp -> 1/D (row form, cheap) ->
                            # PE-broadcast invD and invD^2 to 64 partitions
                            dxt = sc.tile([DH + 1, 512], FP, tag="sc",
                                          name="dxt")
                            nc.vector.tensor_copy(dxt[DH:DH + 1, :],
                                                  cmps[DH:DH + 1, :])
                            nc.vector.tensor_scalar(dxt[DH:DH + 1, :],
                                                    dxt[DH:DH + 1, :],
                                                    1e-15, None, A.max)
                            invb = dwp.tile([DH + 1, 1024], FP, tag="dw",
                                            name="invb")
                            nc.vector.reciprocal(invb[DH:DH + 1, 0:512],
                                                 dxt[DH:DH + 1, :])
                            nc.vector.tensor_tensor(invb[DH:DH + 1, 512:],
                                                    invb[DH:DH + 1, 0:512],
                                                    invb[DH:DH + 1, 0:512],
                                                    A.mult)
                            invf = dwp.tile([DH + 1, 1024], FR, tag="dw",
                                            name="invf")
                            nc.vector.tensor_copy(invf[DH:DH + 1, :],
                                                  invb[DH:DH + 1, :])
                            ibp = ppt.tile([64, 512], FP, tag="pt",
                                           name="ibp")
                            nc.tensor.matmul(ibp, ones65[64:65, 0:64],
                                             invf[DH:DH + 1, 0:512],
                                             start=True, stop=True)
                            ibc = sc.tile([64, 512], FP, tag="sc", name="ibc")
                            nc.scalar.copy(ibc, ibp)
                            ibp2 = ppt.tile([64, 512], FP, tag="pt",
                                            name="ibp2")
                            nc.tensor.matmul(ibp2, ones65[64:65, 0:64],
                                             invf[DH:DH + 1, 512:],
                                             start=True, stop=True)
                            ibc2 = sc.tile([64, 512], FP, tag="sc",
                                           name="ibc2")
                            nc.scalar.copy(ibc2, ibp2)
                            # ctx eviction + normalization -> DRAM scratch
                            fsl = slice(m * 128 + p * 64, m * 128 + p * 64 + 64)
                            tsl = slice(ih * 512, ih * 512 + 512)
                            cm_sb = sc.tile([64, 512], FP, tag="sc",
                                            name="cm_sb")
                            nc.vector.tensor_tensor(cm_sb, cmps[0:DH],
                                                    ibc[0:64], A.mult)
                            nc.sync.dma_start(ctx_dram[b, 0, fsl, tsl], cm_sb)
                            cc_sb = sc.tile([64, 512], FP, tag="sc",
                                            name="cc_sb")
                            nc.vector.tensor_tensor(cc_sb, ccps[0:DH],
                                                    ibc2[0:64], A.mult)
                            nc.sync.dma_start(ctx_dram[b, 1, fsl, tsl], cc_sb)
                            # invD columns for probs normalization
                            icols = tiny.tile([128, 4], FP, tag="ic",
                                              name="icols")
                            for c4 in range(4):
                                nc.sync.dma_start(
                                    icols[:, c4:c4 + 1],
                                    invb[DH:DH + 1,
                                         c4 * 128:(c4 + 1) * 128])
                            # transpose P~t -> natural probs, normalize, store
                            for jq in range(2):
                                for c4 in range(4):
                                    ptp = ppt.tile([128, 512], FP, tag="pt",
                                                   name="ptp")
                                    for jj in range(4):
                                        tTr(ptp[:, jj * 128:(jj + 1) * 128],
                                            pta[:, 4 * jq + jj,
                                                c4 * 128:(c4 + 1) * 128])
                                    pn = sc.tile([128, 512], FP, tag="sc",
                                                 name="pn")
                                    if c4 % 2 == 0:
                                        nc.vector.tensor_scalar(
                                            pn, ptp, icols[:, c4:c4 + 1],
                                            None, A.mult)
                                    else:
                                        nc.scalar.mul(pn, ptp,
                                                      icols[:, c4:c4 + 1])
                                    nc.sync.dma_start(
                                        out_p[b, h,
                                              ih * 512 + c4 * 128:
                                              ih * 512 + (c4 + 1) * 128,
                                              jq * 512:(jq + 1) * 512],
                                        pn)

                # ---------------- dense + layernorm ----------------
                lnw_bc = bcast_row(lnw_d, "lnw_bc")
                lnb_bc = bcast_row(lnb_d, "lnb_bc")
                for path, (wname, xres, outd) in enumerate(
                        (("md", xm, out_mh), ("cd", xc, out_ch))):
                    wd_ = load_w_big(Wd[wname], f"w_{wname}")
                    bb = bcast_row(Bd[wname], f"bb_{wname}")
                    ctxT = big.tile([128, KC, S], FR, tag="big", name="ctxT")
                    ctx_v = ctx_dram[b, path].rearrange(
                        "(ko ki) s -> ki ko s", ki=128)
                    for k in range(KC):
                        cstg = ksqp.tile([128, S], FP, tag="ksq",
                                         name="cstg")
                        nc.sync.dma_start(cstg, ctx_v[:, k, :])
                        nc.gpsimd.tensor_copy(ctxT[:, k, :], cstg)
                    for t_ in range(8):
                        ps = pg.tile([128, 512], FP, tag="g", name="ps_d")
                        for k in range(KC):
                            nc.tensor.matmul(
                                ps, ctxT[:, k, t_ * 128:(t_ + 1) * 128],
                                wd_[:, k, :],
                                start=(k == 0), stop=(k == KC - 1))
                        res = sc.tile([128, 512], FP, tag="sc", name="res")
                        nc.sync.dma_start(
                            res,
                            xres[b * S + t_ * 128: b * S + (t_ + 1) * 128, :])
                        s1 = sc.tile([128, 512], FP, tag="sc", name="s1")
                        nc.vector.tensor_tensor(s1, ps, bb, A.add)
                        nc.vector.tensor_tensor(s1, s1, res, A.add)
                        stats = tiny.tile([128, 6], FP, tag="st",
                                          name="stats")
                        nc.vector.bn_stats(stats, s1)
                        mv2 = tiny.tile([128, 2], FP, tag="mv2", name="mv2")
                        nc.vector.bn_aggr(mv2, stats)
                        std = tiny.tile([128, 1], FP, tag="std", name="std")
                        nc.scalar.activation(std, mv2[:, 1:2], AF.Sqrt,
                                             bias=eps_t)
                        rstd = tiny.tile([128, 1], FP, tag="rstd",
                                         name="rstd")
                        nc.vector.reciprocal(rstd, std)
                        s2 = sc.tile([128, 512], FP, tag="sc", name="s2")
                        nc.vector.tensor_scalar(s2, s1, mv2[:, 0:1], rstd,
                                                A.subtract, A.mult)
                        nc.vector.tensor_tensor(s2, s2, lnw_bc, A.mult)
                        nc.vector.tensor_tensor(s2, s2, lnb_bc, A.add)
                        nc.sync.dma_start(
                            outd[b * S + t_ * 128: b * S + (t_ + 1) * 128, :],
                            s2)
    nc.compile()
    return nc


def _get_program():
    if "nc" not in _CACHE:
        _CACHE["nc"] = _build_program()
    return _CACHE["nc"]


def kernel(**inputs):
    from concourse.bass_utils import run_bass_kernel_spmd

    nc = _get_program()
    in_maps = []
    for c in range(NCORES):
        bsl = slice(c * BL, (c + 1) * BL)
        m = {
            "x_mean": np.ascontiguousarray(
                inputs["x_mean"][bsl].reshape(T, H)),
            "x_resp_mean": np.ascontiguousarray(
                inputs["x_resp_mean"][bsl].reshape(T, H)),
            "x_cov": np.ascontiguousarray(
                inputs["x_cov"][bsl].reshape(T, H)),
            "x_resp_cov": np.ascontiguousarray(
                inputs["x_resp_cov"][bsl].reshape(T, H)),
            "attn_mask": np.ascontiguousarray(
                inputs["attn_mask"].reshape(S, S)),
            "ln_w": inputs["ln_w"], "ln_b": inputs["ln_b"],
        }
        for n in ["mq", "mk", "mv", "cq", "ck", "cv", "md", "cd"]:
            m[n + "_w"] = inputs[n + "_w"]
            m[n + "_b"] = inputs[n + "_b"]
        in_maps.append(m)

    res = run_bass_kernel_spmd(nc, in_maps, list(range(NCORES))).results

    mean_h = np.concatenate(
        [res[c]["mean_h"].reshape(BL, S, H) for c in range(NCORES)], axis=0)
    cov_h = np.concatenate(
        [res[c]["cov_h"].reshape(BL, S, H) for c in range(NCORES)], axis=0)
    probs = np.concatenate(
        [res[c]["probs"] for c in range(NCORES)], axis=0)
    return mean_h, cov_h, probs
